# revision 21
# baseline (speedup 1.0000x reference)
"""MoE (16 experts, top-1 gate, D=H=768) Trainium2 kernel.

Strategy (expert-parallel, per the sharding hint):
  - Host computes the gate (logits argmax) — this IS the dispatch step that
    decides the sharding: tokens are routed to the core owning their expert.
  - 16 experts are sharded 2-per-core across the 8 NeuronCores. Experts are
    sorted by routed-token count: the 8 largest go in slot 0 (capacity C0),
    the 8 smallest in slot 1 (capacity C1 <= C0), so every core does the
    identical padded work and padding waste is minimized.
  - Each core runs the two-GEMM MLP (x @ W1.T -> GELU -> @ W2.T) for its two
    experts over its routed tokens, padded to the slot capacity.
  - Host scatters per-token outputs back to the full [B, N, D] tensor.

Device kernel details (v2 — PE-stream optimized):
  - Matmul operands are fp16 (full PE rate, 1 col/cycle @2.4GHz; fp32 PSUM
    accumulation keeps rel err ~4e-4 end to end).
  - The PE clock ramps (0.65 -> 1.2 -> 2.4 GHz after ~3us of continuous
    execution). N_WARM dummy matmuls on a memset tile lead the PE queue so
    the ramp completes during the DMA lead-in and the real stream runs at
    full clock from its first instruction.
  - MM1 per chunk uses 6 PSUM banks (one per h-chunk) ordered
    dc0x(hc0..5), dc1x(hc0..5), then hc-outer over dc2..5: the first 12
    matmuls touch only the first two 128-row d-pieces of w1/x (tolerating
    DMA arrival latency), and the 6 PSUM completions spread out every 4
    matmuls so the per-hc GELUs (scalar engine, ~634ns each) never bunch.
  - MM2 runs hc-outer in two dc-waves reusing the same 6 banks: all GELU
    outputs are complete before MM2 starts, so the PE streams all 144
    matmuls of the kernel back-to-back with no cross-engine stalls.
  - DMA: the SP HWDGE queue carries the weight stream in consumption order
    (w1s0 pieces, b2s0, w2s0, w1s1, biases, xs1, w2s1); the ACT queue
    carries slot-0 x pieces + b1s0, then output flushes (split across both
    queues so the tail drains fast). All input descriptors are issued
    before any compute instruction so both rings stream continuously.
"""

import json

import ml_dtypes
import numpy as np

import concourse.bass as bass
import concourse.mybir as mybir
import concourse.tile as tile
from concourse.bass_utils import run_bass_kernel_spmd

E = 16          # experts
D = 768         # d_model
H = 768         # d_hidden
NCORES = 8
EPC = E // NCORES   # experts (slots) per core = 2
DC = D // 128       # 6 d-chunks
HC = H // 128       # 6 h-chunks

MM_DTYPE = "f16"   # "f16" | "bf16" | "f32r"
N_WARM = 7         # PE-clock warmup matmuls (512 cols each)

F32 = mybir.dt.float32


def _mm_dt():
    if MM_DTYPE == "f16":
        # fp16 runs at the same PE rate as bf16 (1 col/cycle + FWL weight
        # loads) but has 10 mantissa bits instead of 7 — ~6x lower rounding
        # error. All operands here (|x| < ~6, |W| < ~0.2, GELU outputs) are
        # far inside fp16 range and accumulation is fp32 PSUM.
        return mybir.dt.float16, np.float16
    if MM_DTYPE == "bf16":
        return mybir.dt.bfloat16, ml_dtypes.bfloat16
    return mybir.dt.float32r, np.float32


def _split_multi_waits(nc):
    """Walrus (this image's build) rejects >1 sem-wait on one instruction
    ("Too many sync wait commands" on the TileContext-exit Drain). Move
    excess waits onto a chain of same-engine NoOps directly before the
    instruction — the sequencer runs them in program order, so the
    happens-after relation is preserved exactly."""
    bir = json.loads(nc.to_json_bytes())
    nid = 0
    for fn in bir["functions"]:
        for blk in fn["blocks"]:
            out = []
            for ins in blk["instructions"]:
                si = ins.get("sync_info")
                waits = (si or {}).get("on_wait") or []
                if len(waits) > 1:
                    for w in waits[:-1]:
                        nid += 1
                        out.append({
                            "debug": ins.get("debug", 0),
                            "name": f"I-waitfix{nid}",
                            "opcode": "NoOp",
                            "engine": ins["engine"],
                            "ins": [],
                            "outs": [],
                            "sync_info": {"on_update": [], "on_wait": [w]},
                        })
                    si["on_wait"] = waits[-1:]
                out.append(ins)
            blk["instructions"] = out
    data = json.dumps(bir).encode()
    nc.to_json_bytes = lambda: data
    return nc


def _chunking(C):
    chunks = []
    c0 = 0
    while c0 < C:
        cw = min(512, C - c0)
        chunks.append((c0, cw))
        c0 += cw
    return chunks


def _build(C0, C1):
    """Per-core SPMD kernel: slot 0 with token capacity C0, slot 1 with C1
    (both multiples of 128, >=256). Token dim in chunks of <=512 (PSUM bank
    limit for fp32 accumulation)."""
    caps = [C0, C1]
    slot_chunks = [_chunking(C) for C in caps]

    MMDT, _ = _mm_dt()

    nc = bass.Bass("TRN2", target_bir_lowering=False, debug=False,
                   num_devices=NCORES)
    # Layouts match the SBUF tiles exactly (partition-major) so every DMA is
    # a large contiguous burst.
    xts_d = [nc.dram_tensor(f"xt{s}", [128, DC, caps[s]], MMDT,
                            kind="ExternalInput") for s in range(EPC)]
    # fp16 outputs: halves write traffic (shares HBM with the input
    # stream); |y| <= ~1.3 so fp16 adds only ~2e-4 rel err.
    yts_d = [nc.dram_tensor(f"yt{s}", [128, DC, caps[s]], MMDT,
                            kind="ExternalOutput") for s in range(EPC)]
    w1t = nc.dram_tensor("w1t", [EPC, 128, DC, H], MMDT, kind="ExternalInput")
    w2t = nc.dram_tensor("w2t", [EPC, 128, HC, D], MMDT, kind="ExternalInput")
    b1c = nc.dram_tensor("b1c", [EPC, 128, HC], F32, kind="ExternalInput")
    b2c = nc.dram_tensor("b2c", [EPC, 128, DC], F32, kind="ExternalInput")

    GELU = mybir.ActivationFunctionType.Gelu

    with tile.TileContext(nc) as tc:
        with (
            tc.tile_pool(name="xp", bufs=1) as xp,
            tc.tile_pool(name="wp", bufs=1) as wp,
            tc.tile_pool(name="gp", bufs=2) as gp,
            tc.tile_pool(name="yp", bufs=3) as yp,
            tc.tile_pool(name="bp", bufs=2) as bp,
            tc.tile_pool(name="zp", bufs=1) as zp,
            # PSUM: 6 accumulation banks A0..A5 shared by MM1 (per-hc) and
            # MM2 (per-dc), 1 warmup bank W. All bufs=1 -> bank identity is
            # the tag; cross-use ordering comes from Tile's RAW/WAR deps.
            tc.tile_pool(name="pp", bufs=1, space="PSUM") as pp,
        ):
            # ---- PE warmup: dummy matmuls lead the PE queue so the clock
            # ramp (0.65->1.2->2.4GHz over ~4.5us of continuous execution)
            # overlaps the DMA lead-in instead of eating into the real
            # stream. The operand tile is deliberately uninitialized SBUF —
            # the result lands in a dead PSUM bank and is never read, and
            # skipping the memset starts the PE ~1.3us earlier.
            z = zp.tile([128, 512], MMDT, tag="z", name="z")
            # one-column memset just to give the tile a writer (Tile refuses
            # a never-written tile); the rest stays uninitialized garbage.
            nc.gpsimd.memset(z[:, 0:1], 0.0)
            wps = pp.tile([128, 512], F32, tag="W", name="wps")
            for _ in range(N_WARM):
                nc.tensor.matmul(wps[:, :], z[:, 0:128], z[:, :],
                                 start=True, stop=True)

            # ---- tiles
            tiles = []
            for s in range(EPC):
                w1s = wp.tile([128, DC, H], MMDT, tag=f"w1_{s}",
                              name=f"w1s_{s}")
                w2s = wp.tile([128, HC, D], MMDT, tag=f"w2_{s}",
                              name=f"w2s_{s}")
                xcs = [xp.tile([128, DC, 512], MMDT, tag=f"x_{s}_{ci}",
                               name=f"xc_{s}_{ci}")
                       for ci in range(len(slot_chunks[s]))]
                b1s = bp.tile([128, HC], F32, tag="b1", name=f"b1s_{s}")
                b2s = bp.tile([128, DC], F32, tag="b2", name=f"b2s_{s}")
                tiles.append((w1s, w2s, xcs, b1s, b2s))

            # ---- phase 1: issue ALL input DMAs, split into progressive
            # sub-DMAs sized so each completes ~1us before its consumer
            # even on the slowest core. The SP queue (arbitration-favored)
            # carries the weight stream + the start-critical first x
            # pieces; the ACT queue carries the rest of x, the biases and,
            # later, the output flushes. Whole-tensor DMAs are avoided
            # where the completion semaphore would fire after the first
            # consumer needs the leading slice (a missed cliff = stall +
            # PE clock reset, ~2x the stall cost).
            w1s0, w2s0, xcs0, b1s0, b2s0 = tiles[0]
            w1s1, w2s1, xcs1, b1s1, b2s1 = tiles[1]
            c00, cw0 = slot_chunks[0][0]
            xc00 = xcs0[0]
            # SP: interleaved (w1, x) pieces for dc 0-2, w1 dc3, dc4-5
            for dc in range(3):
                nc.sync.dma_start(w1s0[:, dc:dc + 1],
                                  w1t.ap()[0, :, dc:dc + 1])
                nc.sync.dma_start(xc00[:, dc:dc + 1, :cw0],
                                  xts_d[0].ap()[:, dc:dc + 1, c00:c00 + cw0])
            nc.sync.dma_start(w1s0[:, 3:4], w1t.ap()[0, :, 3:4])
            nc.sync.dma_start(w1s0[:, 4:6], w1t.ap()[0, :, 4:6])
            for h0 in range(0, HC, 2):
                nc.sync.dma_start(w2s0[:, h0:h0 + 2, :],
                                  w2t.ap()[0, :, h0:h0 + 2])
            nc.sync.dma_start(w1s1[:, 0:3], w1t.ap()[1, :, 0:3])
            nc.sync.dma_start(w1s1[:, 3:6], w1t.ap()[1, :, 3:6])
            for h0 in range(0, HC, 2):
                nc.sync.dma_start(w2s1[:, h0:h0 + 2, :],
                                  w2t.ap()[1, :, h0:h0 + 2])
            # ACT: biases (tiny, early), x dc3-5, slot1 x in halves
            nc.scalar.dma_start(b1s0[:, :], b1c.ap()[0])
            nc.scalar.dma_start(b2s0[:, :], b2c.ap()[0])
            nc.scalar.dma_start(b1s1[:, :], b1c.ap()[1])
            nc.scalar.dma_start(b2s1[:, :], b2c.ap()[1])
            nc.scalar.dma_start(xc00[:, 3:6, :cw0],
                                xts_d[0].ap()[:, 3:6, c00:c00 + cw0])
            for ci, (c0, cw) in enumerate(slot_chunks[0][1:], start=1):
                nc.scalar.dma_start(xcs0[ci][:, :, :cw],
                                    xts_d[0].ap()[:, :, c0:c0 + cw])
            for ci, (c0, cw) in enumerate(slot_chunks[1]):
                nc.scalar.dma_start(xcs1[ci][:, 0:3, :cw],
                                    xts_d[1].ap()[:, 0:3, c0:c0 + cw])
                nc.scalar.dma_start(xcs1[ci][:, 3:6, :cw],
                                    xts_d[1].ap()[:, 3:6, c0:c0 + cw])

            # ---- phase 2: compute
            out_flip = [0]

            for s in range(EPC):
                chunks = slot_chunks[s]
                w1s, w2s, xcs, b1s, b2s = tiles[s]
                last_slot = (s == EPC - 1)
                for ci, (c0, cw) in enumerate(chunks):
                    xc = xcs[ci]
                    last_chunk = last_slot and (ci == len(chunks) - 1)

                    # -- MM1: 6 psum banks, DMA-latency-tolerant order
                    ps1 = [pp.tile([128, 512], F32, tag=f"A{h}",
                                   name=f"ps1_{s}_{ci}_{h}")
                           for h in range(HC)]
                    gc = gp.tile([128, HC, 512], MMDT, tag="g",
                                 name=f"gc_{s}_{ci}")

                    # pure dc-outer: pass k touches only the k-th (w1, x)
                    # piece pair, matching the SP queue's FIFO arrival
                    # order exactly. GELU(hc) fires as soon as (dc5, hc)
                    # lands; the scalar cascade (6 x ~634ns) runs only
                    # ~634ns/wave ahead of MM2's hc-waves (3 x 216ns), so
                    # the PE slips < ~0.3us at the MM1->MM2 boundary.
                    for dc in range(DC):
                        for hc in range(HC):
                            nc.tensor.matmul(
                                ps1[hc][:, :cw],
                                w1s[:, dc, hc * 128:(hc + 1) * 128],
                                xc[:, dc, :cw],
                                start=(dc == 0), stop=(dc == DC - 1),
                            )
                            if dc == DC - 1:
                                nc.scalar.activation(
                                    gc[:, hc, :cw], ps1[hc][:, :cw], GELU,
                                    bias=b1s[:, hc:hc + 1], scale=1.0)

                    # -- MM2: hc-outer in two dc-waves over the same banks
                    ps2 = [pp.tile([128, 512], F32, tag=f"A{d}",
                                   name=f"ps2_{s}_{ci}_{d}")
                           for d in range(DC)]
                    for wave in (range(0, 3), range(3, 6)):
                        for hc in range(HC):
                            for dc in wave:
                                nc.tensor.matmul(
                                    ps2[dc][:, :cw],
                                    w2s[:, hc, dc * 128:(dc + 1) * 128],
                                    gc[:, hc, :cw],
                                    start=(hc == 0), stop=(hc == HC - 1),
                                )
                        dl = wave[0]
                        yc = yp.tile([128, 3, 512], MMDT, tag="y",
                                     name=f"yc_{s}_{ci}_{dl}")
                        for dc in wave:
                            nc.vector.tensor_scalar_add(
                                yc[:, dc - dl, :cw], ps2[dc][:, :cw],
                                b2s[:, dc:dc + 1])
                            if last_chunk:
                                # final flush per-dc, alternating queues
                                # (SP queue is empty of inputs by now), so
                                # the tail drains as it is produced
                                eng = (nc.sync, nc.scalar)[out_flip[0] & 1]
                                out_flip[0] += 1
                                eng.dma_start(
                                    yts_d[s].ap()[:, dc, c0:c0 + cw],
                                    yc[:, dc - dl, :cw])
                        if not last_chunk:
                            # mid-kernel outputs ride the ACT queue, which
                            # carries nothing else after the biases — the
                            # SP queue keeps its input stream uninterrupted
                            nc.scalar.dma_start(
                                yts_d[s].ap()[:, dl:dl + 3, c0:c0 + cw],
                                yc[:, :, :cw])

    return _split_multi_waits(nc)


_NC_CACHE = {}


def _get_nc(C0, C1):
    key = (C0, C1, MM_DTYPE, N_WARM)
    nc = _NC_CACHE.get(key)
    if nc is None:
        nc = _build(C0, C1)
        _NC_CACHE[key] = nc
    return nc


def _cap(n):
    # The matmul moving dim is arbitrary, so capacity = exact max routed
    # count (floor 256 guards degenerate routing) — no 128-rounding, which
    # would add ~4% padding FLOPs for nothing.
    return int(max(256, int(n)))


def kernel(x, W1, b1, W2, b2, Wg, bg):
    x = np.ascontiguousarray(np.asarray(x, dtype=np.float32))
    W1 = np.asarray(W1, dtype=np.float32)
    b1 = np.asarray(b1, dtype=np.float32)
    W2 = np.asarray(W2, dtype=np.float32)
    b2 = np.asarray(b2, dtype=np.float32)
    Wg = np.asarray(Wg, dtype=np.float32)
    bg = np.asarray(bg, dtype=np.float32)

    B, N, Dx = x.shape
    assert Dx == D and W1.shape == (E, H, D)
    T = B * N
    t = x.reshape(T, D)

    # --- gate / dispatch (host): this decides the sharding ---
    logits = t @ Wg.T + bg
    idx = np.argmax(logits, axis=1)

    counts = np.bincount(idx, minlength=E)
    # slot 0 <- 8 largest experts, slot 1 <- 8 smallest
    order = np.argsort(-counts, kind="stable")
    slot_experts = [order[:NCORES], order[NCORES:]]
    C0 = _cap(counts[slot_experts[0]].max())
    C1 = _cap(counts[slot_experts[1]].max())
    caps = [C0, C1]
    nc = _get_nc(C0, C1)
    _, npdt = _mm_dt()

    tok_ids = [np.nonzero(idx == e)[0] for e in range(E)]

    # --- host-side layout prep ---
    t_mm = t.astype(npdt)
    # w1t[e, i, dc, h] = W1[e, h, dc*128+i] (partition-major, chunk, col)
    w1t_all = np.ascontiguousarray(
        W1.astype(npdt).transpose(0, 2, 1).reshape(E, DC, 128, H)
        .transpose(0, 2, 1, 3))
    w2t_all = np.ascontiguousarray(
        W2.astype(npdt).transpose(0, 2, 1).reshape(E, HC, 128, D)
        .transpose(0, 2, 1, 3))
    # b1c[e, i, hc] = b1[e, hc*128+i]
    b1c_all = np.ascontiguousarray(b1.reshape(E, HC, 128).transpose(0, 2, 1))
    b2c_all = np.ascontiguousarray(b2.reshape(E, DC, 128).transpose(0, 2, 1))

    in_maps = []
    for c in range(NCORES):
        experts = [int(slot_experts[s][c]) for s in range(EPC)]
        m = {
            "w1t": np.ascontiguousarray(w1t_all[experts]),
            "w2t": np.ascontiguousarray(w2t_all[experts]),
            "b1c": np.ascontiguousarray(b1c_all[experts]),
            "b2c": np.ascontiguousarray(b2c_all[experts]),
        }
        for s in range(EPC):
            C = caps[s]
            xts = np.zeros((128, DC, C), npdt)
            ids = tok_ids[experts[s]]
            n = len(ids)
            if n:
                xts[:, :, :n] = (
                    t_mm[ids].T.reshape(DC, 128, n).transpose(1, 0, 2))
            m[f"xt{s}"] = xts
        in_maps.append(m)

    res = run_bass_kernel_spmd(nc, in_maps, core_ids=list(range(NCORES)))

    out = np.empty((T, D), np.float32)
    for c in range(NCORES):
        for s in range(EPC):
            e = int(slot_experts[s][c])
            ids = tok_ids[e]
            n = len(ids)
            if n:
                yt = res.results[c][f"yt{s}"]  # [128, DC, C]
                out[ids] = yt.transpose(1, 0, 2).reshape(D, caps[s])[:, :n].T
    return out.reshape(B, N, D)
